# revision 8
# baseline (speedup 1.0000x reference)
"""Trainium2 Bass kernel for nn_Capsule (dynamic routing capsule layer).

Math: with cij initialized to zeros, routing iteration 1 collapses to
cij = 1/32 (softmax of zeros), so the whole forward reduces to:
  T[b,j,d]   = sum_n u_hat[b,j,n,d]            (= rowsum(u[b]) @ W)
  S1         = sum(u_hat) = sum(T)
  S2         = sum(u_hat^2) = <W W^T, u^T u>   (feature Gram)
  s          = S1 * rsqrt(max(S2, 1e-12))      (global l2_normalize scalar)
  sjh2       = (s/32) * T ; sj2 = sjh2 * rsqrt(max(sum(sjh2^2), 1e-12))
  logits     = s * (u @ A[b]),  A[b][din,j] = sum_dd W[din,(j,dd)] sj2[b,j,dd]
  cij        = softmax_j(logits)
  G[b][j,:]  = sum_n cij[b,j,n] u[b,n,:]
  out        = squash(s * (G[b] fold W))
u_hat (256 MiB) is never materialized.  Sharding: data-parallel over
batch B (4 per core).  Cross-core reduction (Gram + rowsums -> 3
scalars) and the tiny O(B*J*D*DIN) fold/squash run on the host between
the two launches (in-kernel collectives cost ~65us here, far above the
two-launch overhead).

DMA layout: u rows are re-blocked on the host so each SBUF partition's
DRAM source is one contiguous run (row r of a batch lands at partition
r//32, chunk r%32).  Each row is padded to 132 with a one-hot batch
indicator so a single accumulating matmul chain yields both the Gram
(cols 0:128) and per-batch rowsums (cols 128:132).  The Gram and both
phase-2 matmuls are permutation-invariant to this row order (softmax
rows are independent; the transposed copy uses the same permutation).
Matmul operands are bf16 (fp32 accumulation in PSUM, rel err ~4e-3).
"""

import numpy as np

import concourse.bacc as bacc
import concourse.mybir as mybir
import concourse.tile as tile
from concourse.bass import ts
from concourse.bass_utils import run_bass_kernel_spmd

N_CORES = 8
B, N, DIN = 32, 4096, 128
J, D = 32, 16
K = J * D  # 512
B_LOC = B // N_CORES          # 4 batches per core
CPB = N // 128                # 32 chunks of 128 rows per batch
E1 = DIN + B_LOC              # 132: row + one-hot batch indicator
NH = 2 * B_LOC                # 8 half-batch groups
CPH = CPB // 2                # 16 chunks per half-batch
F32 = mybir.dt.float32
BF16 = mybir.dt.bfloat16
AX = mybir.AxisListType
ALU = mybir.AluOpType
ACTF = mybir.ActivationFunctionType

PROFILE = False
LAST_TIMES = {}

_CACHE = {}


def _new_bass():
    return bacc.Bacc(
        "TRN2",
        target_bir_lowering=False,
        debug=False,
        enable_asserts=False,
        num_devices=N_CORES,
    )


def _build_phase1():
    """Per core: one accumulating matmul chain over 128 row-chunks of
    the padded u layout -> [C | R] = [128, 132] (Gram + per-batch
    rowsums)."""
    nc = _new_bass()
    u_d = nc.dram_tensor("u1", [128, B_LOC * CPB * E1], BF16, kind="ExternalInput")
    o_d = nc.dram_tensor("p1", [128, E1], F32, kind="ExternalOutput")

    with tile.TileContext(nc) as tc:
        with (
            tc.tile_pool(name="upool", bufs=1) as upool,
            tc.tile_pool(name="psp", bufs=1, space="PSUM") as psp,
            tc.tile_pool(name="sbp", bufs=1) as sbp,
        ):
            # one 1.03 MiB DMA per batch, spread over the three DMA
            # issuers (2 HWDGE rings + SWDGE); each partition's DRAM
            # source is one contiguous 32*132*2 B run.
            dma_engs = [nc.sync, nc.scalar, nc.gpsimd, nc.sync]
            ugs = []
            for b in range(B_LOC):
                ug = upool.tile([128, CPB * E1], BF16, tag=f"ug{b}", name=f"ug{b}")
                ugs.append(ug)
                dma_engs[b].dma_start(ug[:], u_d.ap()[:, ts(b, CPB * E1)])

            acc = psp.tile([128, E1], F32, tag="acc", name="acc")
            for c in range(B_LOC * CPB):
                b, cl = divmod(c, CPB)
                view = ugs[b][:].rearrange("p (c e) -> p c e", e=E1)[:, cl, :]
                nc.tensor.matmul(
                    acc[:],
                    view[:, 0:DIN],
                    view,
                    start=(c == 0),
                    stop=(c == B_LOC * CPB - 1),
                )

            outsb = sbp.tile([128, E1], F32, tag="outsb", name="outsb")
            nc.scalar.copy(outsb[:], acc[:])
            nc.sync.dma_start(o_d.ap(), outsb[:])

    nc.compile()
    return nc


PKW = CPH * 128 + CPH * E1  # 4160: [ut_h | u1_h] packed half-batch width


def _build_phase2():
    """Per core: logits -> softmax -> G accumulation; returns G."""
    nc = _new_bass()
    # packed per half-batch: cols [0:2048] = transposed u, cols
    # [2048:4160] = natural u rows padded to 132
    u_d = nc.dram_tensor("pk", [128, NH * PKW], BF16, kind="ExternalInput")
    a_d = nc.dram_tensor("A", [DIN, B_LOC * J], BF16, kind="ExternalInput")  # s*A
    # out row 32*b+j holds G[b, j, :] (length-128 din)
    o_d = nc.dram_tensor("out", [128, DIN], F32, kind="ExternalOutput")

    with tile.TileContext(nc) as tc:
        with (
            tc.tile_pool(name="const", bufs=1) as cstp,
            tc.tile_pool(name="upool", bufs=1) as upool,
            tc.tile_pool(name="expp", bufs=2) as expp,
            tc.tile_pool(name="cijp", bufs=3) as cijp,
            tc.tile_pool(name="zp", bufs=2) as zp,
            tc.tile_pool(name="sbt", bufs=1) as sbt,
            tc.tile_pool(name="plp", bufs=4, space="PSUM") as plp,
            tc.tile_pool(name="tlp", bufs=1, space="PSUM") as tlp,
        ):
            # small load first so it doesn't queue behind the u loads
            a_sb = cstp.tile([128, B_LOC * J], BF16, tag="a_sb", name="a_sb")
            nc.sync.dma_start(a_sb[:], a_d.ap())

            # one 1.04 MiB packed DMA per half-batch over 3 issuers
            dma_engs = [nc.sync, nc.scalar, nc.gpsimd, nc.sync,
                        nc.scalar, nc.gpsimd, nc.sync, nc.scalar]
            pks = []
            for h in range(NH):
                pk = upool.tile([128, PKW], BF16, tag=f"pk{h}", name=f"pk{h}")
                pks.append(pk)
                dma_engs[h].dma_start(pk[:], u_d.ap()[:, ts(h, PKW)])

            psg = tlp.tile([128, DIN], F32, tag="psg", name="psg")  # G accumulator

            pls = [None] * NH
            LAG = 3  # half-batches of logits emitted ahead of their chain

            def emit_logits(h):
                b = h // 2
                pls[h] = plp.tile([128, CPH * J], F32, tag="pl", name=f"pl{h}")
                for cl in range(CPH):
                    nc.tensor.matmul(
                        pls[h][:, ts(cl, J)],
                        pks[h][:, ts(cl, 128)],
                        a_sb[:, ts(b, J)],
                        start=True,
                        stop=True,
                    )

            def emit_chain(h):
                # softmax over j (free axis) + G matmuls for half-batch h
                b = h // 2
                eg = expp.tile([128, CPH * J], F32, tag="eg", name=f"eg{h}")
                nc.scalar.activation(eg[:], pls[h][:], ACTF.Exp)
                zg = zp.tile([128, CPH], F32, tag="zg", name=f"zg{h}")
                nc.vector.reduce_sum(
                    zg[:], eg[:].rearrange("p (c j) -> p c j", j=J), axis=AX.X
                )
                zr = zp.tile([128, CPH], F32, tag="zr", name=f"zr{h}")
                nc.vector.reciprocal(zr[:], zg[:])
                cg = cijp.tile([128, CPH * J], BF16, tag="cg", name=f"cg{h}")
                nc.vector.tensor_tensor(
                    cg[:].rearrange("p (c j) -> p c j", j=J),
                    eg[:].rearrange("p (c j) -> p c j", j=J),
                    zr[:].unsqueeze(2).broadcast_to([128, CPH, J]),
                    op=ALU.mult,
                )
                for cl in range(CPH):
                    c = h * CPH + cl
                    off = CPH * 128 + cl * E1
                    nc.tensor.matmul(
                        psg[ts(b, J), :],
                        cg[:, ts(cl, J)],
                        pks[h][:, off : off + DIN],
                        start=(c % CPB == 0),
                        stop=(c % CPB == CPB - 1),
                        tile_position=(0, J * b),
                    )

            for h in range(NH):
                emit_logits(h)
                if h >= LAG:
                    emit_chain(h - LAG)
            for h in range(NH - LAG, NH):
                emit_chain(h)

            gout = sbt.tile([128, DIN], F32, tag="gout", name="gout")
            nc.scalar.copy(gout[:], psg[:])
            nc.sync.dma_start(o_d.ap(), gout[:])

    nc.compile()
    return nc


def _get(name):
    if name not in _CACHE:
        if name == "p1":
            _CACHE[name] = _build_phase1()
        else:
            _CACHE[name] = _build_phase2()
    return _CACHE[name]


def kernel(u, W):
    import ml_dtypes

    bf16 = ml_dtypes.bfloat16
    u = np.ascontiguousarray(u, dtype=np.float32)
    W = np.ascontiguousarray(W, dtype=np.float32)
    W0 = np.ascontiguousarray(W[0])  # [128, 512]
    ub = u.astype(bf16)

    # padded re-blocked layout: u1[i][p, ((b,c),e)] = [u[4i+b, 32p+c, :] | e_b]
    up = np.zeros((B, N, E1), dtype=bf16)
    up[:, :, :DIN] = ub
    for b in range(B_LOC):
        up[b::B_LOC, :, DIN + b] = 1.0  # batch index within the core shard
    up = up.reshape(N_CORES, B_LOC, 128, CPB, E1).transpose(0, 2, 1, 3, 4)
    u1 = [np.ascontiguousarray(up[i].reshape(128, B_LOC * CPB * E1))
          for i in range(N_CORES)]
    # transposed copy with the same row permutation:
    # ut[i][d, b*4096 + c*128 + p] = u[4i+b, 32p+c, d]
    ut3 = ub.reshape(N_CORES, B_LOC, 128, CPB, DIN).transpose(0, 4, 1, 3, 2)
    # phase-2 packed layout: per half-batch h, [ut_h | u1_h]
    pk = np.empty((N_CORES, 128, NH, PKW), dtype=bf16)
    utv = ut3.reshape(N_CORES, 128, NH, CPH * 128)
    u1v = up.reshape(N_CORES, 128, NH, CPH * E1)
    pk[:, :, :, : CPH * 128] = utv
    pk[:, :, :, CPH * 128 :] = u1v
    pks = [np.ascontiguousarray(pk[i].reshape(128, NH * PKW))
           for i in range(N_CORES)]

    # ---- phase 1: per-core Gram + rowsums ----
    nc1 = _get("p1")
    r1 = run_bass_kernel_spmd(
        nc1,
        [{"u1": u1[i]} for i in range(N_CORES)],
        core_ids=list(range(N_CORES)),
        trace=PROFILE,
    )
    if PROFILE:
        LAST_TIMES["phase1_ns"] = r1.exec_time_ns

    # ---- host: global scalar reduction (the "all-reduce" of 3 scalars) ----
    C = np.zeros((128, 128), dtype=np.float64)
    Rall = np.empty((128, B), dtype=np.float64)
    for i in range(N_CORES):
        p = r1.results[i]["p1"].astype(np.float64)
        C += p[:, :DIN]
        Rall[:, i * B_LOC : (i + 1) * B_LOC] = p[:, DIN:E1]
    W0d = W0.astype(np.float64)
    M = W0d @ W0d.T
    S2 = float(np.vdot(M, C))
    T = Rall.T @ W0d  # [B, 512]
    S1 = float(T.sum())
    s = S1 / np.sqrt(max(S2, 1e-12))
    sjh2 = (s / J) * T
    n2 = float((sjh2 * sjh2).sum())
    sj2 = (sjh2 / np.sqrt(max(n2, 1e-12))).reshape(B, J, D)
    # A[b][din, j] = sum_dd W0[din, j*16+dd] * sj2[b, j, dd];  fold s in
    A = np.einsum("dje,bje->bdj", W0d.reshape(DIN, J, D), sj2)
    As = (s * A).astype(bf16)  # [B, 128, 32]

    # ---- phase 2: logits/softmax/G ----
    nc2 = _get("p2")
    in2 = [
        {
            "pk": pks[i],
            "A": np.ascontiguousarray(
                As[i * B_LOC : (i + 1) * B_LOC].transpose(1, 0, 2).reshape(DIN, -1)
            ),
        }
        for i in range(N_CORES)
    ]
    r2 = run_bass_kernel_spmd(
        nc2, in2, core_ids=list(range(N_CORES)), trace=PROFILE
    )
    if PROFILE:
        LAST_TIMES["phase2_ns"] = r2.exec_time_ns

    # ---- host: tiny fold + squash (O(B*J*D*DIN)) ----
    G = np.concatenate(
        [r2.results[i]["out"].astype(np.float64).reshape(B_LOC, J, DIN)
         for i in range(N_CORES)]
    )  # [B, J, 128]
    sjh3 = s * np.einsum("bjd,dje->bje", G, W0d.reshape(DIN, J, D))
    s2 = (sjh3 * sjh3).sum(axis=-1, keepdims=True) + 1e-7
    out = (np.sqrt(s2) / (1.0 + s2)) * sjh3
    return out.astype(np.float32)


# revision 10
# speedup vs baseline: 1.0155x; 1.0155x over previous
"""Trainium2 Bass kernel for nn_Capsule (dynamic routing capsule layer).

Math: with cij initialized to zeros, routing iteration 1 collapses to
cij = 1/32 (softmax of zeros), so the whole forward reduces to:
  T[b,j,d]   = sum_n u_hat[b,j,n,d]            (= rowsum(u[b]) @ W)
  S1         = sum(u_hat) = sum(T)
  S2         = sum(u_hat^2) = <W W^T, u^T u>   (feature Gram)
  s          = S1 * rsqrt(max(S2, 1e-12))      (global l2_normalize scalar)
  sjh2       = (s/32) * T ; sj2 = sjh2 * rsqrt(max(sum(sjh2^2), 1e-12))
  logits     = s * (u @ A[b]),  A[b][din,j] = sum_dd W[din,(j,dd)] sj2[b,j,dd]
  cij        = softmax_j(logits)
  G[b][j,:]  = sum_n cij[b,j,n] u[b,n,:]
  out        = squash(s * (G[b] fold W))
u_hat (256 MiB) is never materialized.  Sharding: data-parallel over
batch B (4 per core).  Cross-core reduction (Gram + rowsums -> 3
scalars) and the tiny O(B*J*D*DIN) fold/squash run on the host between
the two launches (in-kernel collectives cost ~65us here, far above the
two-launch overhead).

DMA layout: u rows are re-blocked on the host so each SBUF partition's
DRAM source is one contiguous run (row r of a batch lands at partition
r//32, chunk r%32).  Each row is padded to 132 with a one-hot batch
indicator so a single accumulating matmul chain yields both the Gram
(cols 0:128) and per-batch rowsums (cols 128:132).  The Gram and both
phase-2 matmuls are permutation-invariant to this row order (softmax
rows are independent; the transposed copy uses the same permutation).
Matmul operands are bf16 (fp32 accumulation in PSUM, rel err ~4e-3).
"""

import numpy as np

import concourse.bacc as bacc
import concourse.mybir as mybir
import concourse.tile as tile
from concourse.bass import ts
from concourse.bass_utils import run_bass_kernel_spmd

N_CORES = 8
B, N, DIN = 32, 4096, 128
J, D = 32, 16
K = J * D  # 512
B_LOC = B // N_CORES          # 4 batches per core
CPB = N // 128                # 32 chunks of 128 rows per batch
E1 = DIN + B_LOC              # 132: row + one-hot batch indicator
NH = 2 * B_LOC                # 8 half-batch groups
CPH = CPB // 2                # 16 chunks per half-batch
F32 = mybir.dt.float32
BF16 = mybir.dt.bfloat16
AX = mybir.AxisListType
ALU = mybir.AluOpType
ACTF = mybir.ActivationFunctionType

PROFILE = False
LAST_TIMES = {}

_CACHE = {}


def _new_bass():
    return bacc.Bacc(
        "TRN2",
        target_bir_lowering=False,
        debug=False,
        enable_asserts=False,
        num_devices=N_CORES,
    )


def _build_phase1():
    """Per core: one accumulating matmul chain over 128 row-chunks of
    the padded u layout -> [C | R] = [128, 132] (Gram + per-batch
    rowsums)."""
    nc = _new_bass()
    u_d = nc.dram_tensor("u1", [128, B_LOC * CPB * E1], BF16, kind="ExternalInput")
    o_d = nc.dram_tensor("p1", [128, E1], F32, kind="ExternalOutput")

    with tile.TileContext(nc) as tc:
        with (
            tc.tile_pool(name="upool", bufs=1) as upool,
            tc.tile_pool(name="psp", bufs=1, space="PSUM") as psp,
            tc.tile_pool(name="sbp", bufs=1) as sbp,
        ):
            # 8 half-batch DMAs on the two HWDGE rings; each partition's
            # DRAM source is one contiguous 16*132*2 B run.
            ugs = []
            for h in range(NH):
                ug = upool.tile([128, CPH * E1], BF16, tag=f"ug{h}", name=f"ug{h}")
                ugs.append(ug)
                eng = nc.sync if h % 2 == 0 else nc.scalar
                eng.dma_start(ug[:], u_d.ap()[:, ts(h, CPH * E1)])

            acc = psp.tile([128, E1], F32, tag="acc", name="acc")
            for c in range(B_LOC * CPB):
                h, cl = divmod(c, CPH)
                view = ugs[h][:].rearrange("p (c e) -> p c e", e=E1)[:, cl, :]
                nc.tensor.matmul(
                    acc[:],
                    view[:, 0:DIN],
                    view,
                    start=(c == 0),
                    stop=(c == B_LOC * CPB - 1),
                )

            outsb = sbp.tile([128, E1], F32, tag="outsb", name="outsb")
            nc.scalar.copy(outsb[:], acc[:])
            nc.sync.dma_start(o_d.ap(), outsb[:])

    nc.compile()
    return nc


PKW = CPH * 128 + CPH * E1  # 4160: [ut_h | u1_h] packed half-batch width


def _build_phase2():
    """Per core: logits -> softmax -> G accumulation; returns G."""
    nc = _new_bass()
    # packed per half-batch: cols [0:2048] = transposed u, cols
    # [2048:4160] = natural u rows padded to 132
    u_d = nc.dram_tensor("pk", [128, NH * PKW], BF16, kind="ExternalInput")
    a_d = nc.dram_tensor("A", [DIN, B_LOC * J], BF16, kind="ExternalInput")  # s*A
    # out row 32*b+j holds G[b, j, :] (length-128 din)
    o_d = nc.dram_tensor("out", [128, DIN], F32, kind="ExternalOutput")

    with tile.TileContext(nc) as tc:
        with (
            tc.tile_pool(name="const", bufs=1) as cstp,
            tc.tile_pool(name="upool", bufs=1) as upool,
            tc.tile_pool(name="expp", bufs=2) as expp,
            tc.tile_pool(name="cijp", bufs=3) as cijp,
            tc.tile_pool(name="zp", bufs=2) as zp,
            tc.tile_pool(name="sbt", bufs=1) as sbt,
            tc.tile_pool(name="plp", bufs=4, space="PSUM") as plp,
            tc.tile_pool(name="tlp", bufs=1, space="PSUM") as tlp,
        ):
            # small load first so it doesn't queue behind the u loads
            a_sb = cstp.tile([128, B_LOC * J], BF16, tag="a_sb", name="a_sb")
            nc.sync.dma_start(a_sb[:], a_d.ap())

            # one 1.04 MiB packed DMA per half-batch, ALL on the sync
            # ring: the ring-full DIRECT2D stalls then land on the sync
            # sequencer (idle anyway) instead of blocking ACT's exp chain
            pks = []
            for h in range(NH):
                pk = upool.tile([128, PKW], BF16, tag=f"pk{h}", name=f"pk{h}")
                pks.append(pk)
                nc.sync.dma_start(pk[:], u_d.ap()[:, ts(h, PKW)])

            psg = tlp.tile([128, DIN], F32, tag="psg", name="psg")  # G accumulator

            pls = [None] * NH
            LAG = 3  # half-batches of logits emitted ahead of their chain

            def emit_logits(h):
                b = h // 2
                pls[h] = plp.tile([128, CPH * J], F32, tag="pl", name=f"pl{h}")
                for cl in range(CPH):
                    nc.tensor.matmul(
                        pls[h][:, ts(cl, J)],
                        pks[h][:, ts(cl, 128)],
                        a_sb[:, ts(b, J)],
                        start=True,
                        stop=True,
                    )

            def emit_chain(h):
                # softmax over j (free axis) + G matmuls for half-batch h
                b = h // 2
                eg = expp.tile([128, CPH * J], F32, tag="eg", name=f"eg{h}")
                nc.scalar.activation(eg[:], pls[h][:], ACTF.Exp)
                zg = zp.tile([128, CPH], F32, tag="zg", name=f"zg{h}")
                nc.vector.reduce_sum(
                    zg[:], eg[:].rearrange("p (c j) -> p c j", j=J), axis=AX.X
                )
                zr = zp.tile([128, CPH], F32, tag="zr", name=f"zr{h}")
                nc.vector.reciprocal(zr[:], zg[:])
                cg = cijp.tile([128, CPH * J], BF16, tag="cg", name=f"cg{h}")
                nc.vector.tensor_tensor(
                    cg[:].rearrange("p (c j) -> p c j", j=J),
                    eg[:].rearrange("p (c j) -> p c j", j=J),
                    zr[:].unsqueeze(2).broadcast_to([128, CPH, J]),
                    op=ALU.mult,
                )
                for cl in range(CPH):
                    c = h * CPH + cl
                    off = CPH * 128 + cl * E1
                    nc.tensor.matmul(
                        psg[ts(b, J), :],
                        cg[:, ts(cl, J)],
                        pks[h][:, off : off + DIN],
                        start=(c % CPB == 0),
                        stop=(c % CPB == CPB - 1),
                        tile_position=(0, J * b),
                    )

            for h in range(NH):
                emit_logits(h)
                if h >= LAG:
                    emit_chain(h - LAG)
            for h in range(NH - LAG, NH):
                emit_chain(h)

            gout = sbt.tile([128, DIN], F32, tag="gout", name="gout")
            nc.scalar.copy(gout[:], psg[:])
            nc.sync.dma_start(o_d.ap(), gout[:])

    nc.compile()
    return nc


def _get(name):
    if name not in _CACHE:
        if name == "p1":
            _CACHE[name] = _build_phase1()
        else:
            _CACHE[name] = _build_phase2()
    return _CACHE[name]


def kernel(u, W):
    import ml_dtypes

    bf16 = ml_dtypes.bfloat16
    u = np.ascontiguousarray(u, dtype=np.float32)
    W = np.ascontiguousarray(W, dtype=np.float32)
    W0 = np.ascontiguousarray(W[0])  # [128, 512]
    ub = u.astype(bf16)

    # padded re-blocked layout: u1[i][p, ((b,c),e)] = [u[4i+b, 32p+c, :] | e_b]
    up = np.zeros((B, N, E1), dtype=bf16)
    up[:, :, :DIN] = ub
    for b in range(B_LOC):
        up[b::B_LOC, :, DIN + b] = 1.0  # batch index within the core shard
    up = up.reshape(N_CORES, B_LOC, 128, CPB, E1).transpose(0, 2, 1, 3, 4)
    u1 = [np.ascontiguousarray(up[i].reshape(128, B_LOC * CPB * E1))
          for i in range(N_CORES)]
    # transposed copy with the same row permutation:
    # ut[i][d, b*4096 + c*128 + p] = u[4i+b, 32p+c, d]
    ut3 = ub.reshape(N_CORES, B_LOC, 128, CPB, DIN).transpose(0, 4, 1, 3, 2)
    # phase-2 packed layout: per half-batch h, [ut_h | u1_h]
    pk = np.empty((N_CORES, 128, NH, PKW), dtype=bf16)
    utv = ut3.reshape(N_CORES, 128, NH, CPH * 128)
    u1v = up.reshape(N_CORES, 128, NH, CPH * E1)
    pk[:, :, :, : CPH * 128] = utv
    pk[:, :, :, CPH * 128 :] = u1v
    pks = [np.ascontiguousarray(pk[i].reshape(128, NH * PKW))
           for i in range(N_CORES)]

    # ---- phase 1: per-core Gram + rowsums ----
    nc1 = _get("p1")
    r1 = run_bass_kernel_spmd(
        nc1,
        [{"u1": u1[i]} for i in range(N_CORES)],
        core_ids=list(range(N_CORES)),
        trace=PROFILE,
    )
    if PROFILE:
        LAST_TIMES["phase1_ns"] = r1.exec_time_ns

    # ---- host: global scalar reduction (the "all-reduce" of 3 scalars) ----
    C = np.zeros((128, 128), dtype=np.float64)
    Rall = np.empty((128, B), dtype=np.float64)
    for i in range(N_CORES):
        p = r1.results[i]["p1"].astype(np.float64)
        C += p[:, :DIN]
        Rall[:, i * B_LOC : (i + 1) * B_LOC] = p[:, DIN:E1]
    W0d = W0.astype(np.float64)
    M = W0d @ W0d.T
    S2 = float(np.vdot(M, C))
    T = Rall.T @ W0d  # [B, 512]
    S1 = float(T.sum())
    s = S1 / np.sqrt(max(S2, 1e-12))
    sjh2 = (s / J) * T
    n2 = float((sjh2 * sjh2).sum())
    sj2 = (sjh2 / np.sqrt(max(n2, 1e-12))).reshape(B, J, D)
    # A[b][din, j] = sum_dd W0[din, j*16+dd] * sj2[b, j, dd];  fold s in
    A = np.einsum("dje,bje->bdj", W0d.reshape(DIN, J, D), sj2)
    As = (s * A).astype(bf16)  # [B, 128, 32]

    # ---- phase 2: logits/softmax/G ----
    nc2 = _get("p2")
    in2 = [
        {
            "pk": pks[i],
            "A": np.ascontiguousarray(
                As[i * B_LOC : (i + 1) * B_LOC].transpose(1, 0, 2).reshape(DIN, -1)
            ),
        }
        for i in range(N_CORES)
    ]
    r2 = run_bass_kernel_spmd(
        nc2, in2, core_ids=list(range(N_CORES)), trace=PROFILE
    )
    if PROFILE:
        LAST_TIMES["phase2_ns"] = r2.exec_time_ns

    # ---- host: tiny fold + squash (O(B*J*D*DIN)) ----
    G = np.concatenate(
        [r2.results[i]["out"].astype(np.float64).reshape(B_LOC, J, DIN)
         for i in range(N_CORES)]
    )  # [B, J, 128]
    sjh3 = s * np.einsum("bjd,dje->bje", G, W0d.reshape(DIN, J, D))
    s2 = (sjh3 * sjh3).sum(axis=-1, keepdims=True) + 1e-7
    out = (np.sqrt(s2) / (1.0 + s2)) * sjh3
    return out.astype(np.float32)


# revision 14
# speedup vs baseline: 1.1027x; 1.0858x over previous
"""Trainium2 Bass kernel for nn_Capsule (dynamic routing capsule layer).

Math: with cij initialized to zeros, routing iteration 1 collapses to
cij = 1/32 (softmax of zeros), so the whole forward reduces to:
  T[b,j,d]   = sum_n u_hat[b,j,n,d]            (= rowsum(u[b]) @ W)
  S1         = sum(u_hat) = sum(T)
  S2         = sum(u_hat^2) = <W W^T, u^T u>   (feature Gram)
  s          = S1 * rsqrt(max(S2, 1e-12))      (global l2_normalize scalar)
  sjh2       = (s/32) * T ; sj2 = sjh2 * rsqrt(max(sum(sjh2^2), 1e-12))
  logits     = s * (u @ A[b]),  A[b][din,j] = sum_dd W[din,(j,dd)] sj2[b,j,dd]
  cij        = softmax_j(logits)
  G[b][j,:]  = sum_n cij[b,j,n] u[b,n,:]
  out        = squash(s * (G[b] fold W))
u_hat (256 MiB) is never materialized.  Sharding: data-parallel over
batch B (4 per core).  Cross-core reduction (Gram + rowsums -> 3
scalars) and the tiny O(B*J*D*DIN) fold/squash run on the host between
the two launches (in-kernel collectives cost ~65us here, far above the
two-launch overhead).

DMA layout: u rows are re-blocked on the host so each SBUF partition's
DRAM source is one contiguous run (row r of a batch lands at partition
r//32, chunk r%32).  Each row is padded to 132 with a one-hot batch
indicator so a single accumulating matmul chain yields both the Gram
(cols 0:128) and per-batch rowsums (cols 128:132).  The Gram and both
phase-2 matmuls are permutation-invariant to this row order (softmax
rows are independent; the transposed copy uses the same permutation).
Matmul operands are bf16 (fp32 accumulation in PSUM, rel err ~4e-3).
"""

import numpy as np

import concourse.bacc as bacc
import concourse.mybir as mybir
import concourse.tile as tile
from concourse.bass import ts
from concourse.bass_utils import run_bass_kernel_spmd

N_CORES = 8
B, N, DIN = 32, 4096, 128
J, D = 32, 16
K = J * D  # 512
B_LOC = B // N_CORES          # 4 batches per core
CPB = N // 128                # 32 chunks of 128 rows per batch
E1 = DIN + B_LOC              # 132: row + one-hot batch indicator
NH = 2 * B_LOC                # 8 half-batch groups
CPH = CPB // 2                # 16 chunks per half-batch
F32 = mybir.dt.float32
BF16 = mybir.dt.bfloat16
AX = mybir.AxisListType
ALU = mybir.AluOpType
ACTF = mybir.ActivationFunctionType

PROFILE = False
LAST_TIMES = {}

_CACHE = {}


def _new_bass():
    return bacc.Bacc(
        "TRN2",
        target_bir_lowering=False,
        debug=False,
        enable_asserts=False,
        num_devices=N_CORES,
    )


def _build_phase1():
    """Per core: one accumulating matmul chain over 128 row-chunks of
    the padded u layout -> [C | R] = [128, 132] (Gram + per-batch
    rowsums)."""
    nc = _new_bass()
    u_d = nc.dram_tensor("u1", [128, B_LOC * CPB * E1], BF16, kind="ExternalInput")
    o_d = nc.dram_tensor("p1", [128, E1], F32, kind="ExternalOutput")

    with tile.TileContext(nc) as tc:
        with (
            tc.tile_pool(name="upool", bufs=1) as upool,
            tc.tile_pool(name="psp", bufs=1, space="PSUM") as psp,
            tc.tile_pool(name="sbp", bufs=1) as sbp,
        ):
            # 8 half-batch DMAs on the two HWDGE rings; each partition's
            # DRAM source is one contiguous 16*132*2 B run.
            ugs = []
            for h in range(NH):
                ug = upool.tile([128, CPH * E1], BF16, tag=f"ug{h}", name=f"ug{h}")
                ugs.append(ug)
                eng = nc.sync if h % 2 == 0 else nc.scalar
                eng.dma_start(ug[:], u_d.ap()[:, ts(h, CPH * E1)])

            acc = psp.tile([128, E1], F32, tag="acc", name="acc")
            for c in range(B_LOC * CPB):
                h, cl = divmod(c, CPH)
                view = ugs[h][:].rearrange("p (c e) -> p c e", e=E1)[:, cl, :]
                nc.tensor.matmul(
                    acc[:],
                    view[:, 0:DIN],
                    view,
                    start=(c == 0),
                    stop=(c == B_LOC * CPB - 1),
                )

            outsb = sbp.tile([128, E1], F32, tag="outsb", name="outsb")
            nc.scalar.copy(outsb[:], acc[:])
            nc.sync.dma_start(o_d.ap(), outsb[:])

    nc.compile()
    return nc


PKW = CPH * 128 + CPH * E1  # 4160: [ut_h | u1_h] packed half-batch width


def _build_phase2():
    """Per core: logits -> softmax -> G accumulation; returns G."""
    nc = _new_bass()
    # packed per half-batch: cols [0:2048] = transposed u, cols
    # [2048:4160] = natural u rows padded to 132
    u_d = nc.dram_tensor("pk", [128, NH * PKW], BF16, kind="ExternalInput")
    a_d = nc.dram_tensor("A", [DIN, B_LOC * J], BF16, kind="ExternalInput")  # s*A
    # out row 32*b+j holds G[b, j, :] (length-128 din)
    o_d = nc.dram_tensor("out", [128, DIN], F32, kind="ExternalOutput")

    with tile.TileContext(nc) as tc:
        with (
            tc.tile_pool(name="const", bufs=1) as cstp,
            tc.tile_pool(name="upool", bufs=1) as upool,
            tc.tile_pool(name="expp", bufs=3) as expp,
            tc.tile_pool(name="cijp", bufs=3) as cijp,
            tc.tile_pool(name="zgp", bufs=2) as zgp,
            tc.tile_pool(name="zrp", bufs=2) as zrp,
            tc.tile_pool(name="sbt", bufs=1) as sbt,
            tc.tile_pool(name="plp", bufs=4, space="PSUM") as plp,
            tc.tile_pool(name="tlp", bufs=1, space="PSUM") as tlp,
        ):
            # small load first so it doesn't queue behind the u loads
            a_sb = cstp.tile([128, B_LOC * J], BF16, tag="a_sb", name="a_sb")
            nc.sync.dma_start(a_sb[:], a_d.ap())

            # one 1.04 MiB packed DMA per half-batch.  The ACT (scalar)
            # ring gets only the first two, so its sequencer is free to
            # run the exp chain instead of stalling in ring-full
            # DIRECT2D issue; sync (no compute) carries the rest.
            pks = []
            for h in range(NH):
                pk = upool.tile([128, PKW], BF16, tag=f"pk{h}", name=f"pk{h}")
                pks.append(pk)
                eng = nc.scalar if h < 2 else nc.sync
                eng.dma_start(pk[:], u_d.ap()[:, ts(h, PKW)])

            psg = tlp.tile([128, DIN], F32, tag="psg", name="psg")  # G accumulator

            pls = [None] * NH
            # half-batches of logits emitted ahead of their chain.  DMA
            # arrivals are ~2.7us apart and a chain takes ~3.3us after
            # its tile lands, so chain h is ready just after logits h+1:
            # interleave the static PE stream accordingly (larger LAG
            # makes chain h's G matmuls wait for logits h+LAG's DMA).
            LAG = 1

            def emit_logits(h):
                b = h // 2
                pls[h] = plp.tile([128, CPH * J], F32, tag="pl", name=f"pl{h}")
                for cl in range(CPH):
                    nc.tensor.matmul(
                        pls[h][:, ts(cl, J)],
                        pks[h][:, ts(cl, 128)],
                        a_sb[:, ts(b, J)],
                        start=True,
                        stop=True,
                    )

            def emit_chain(h):
                # softmax over j (free axis) + G matmuls for half-batch h
                b = h // 2
                eg = expp.tile([128, CPH * J], F32, tag="eg", name=f"eg{h}")
                nc.scalar.activation(eg[:], pls[h][:], ACTF.Exp)
                zg = zgp.tile([128, CPH], F32, tag="zg", name=f"zg{h}")
                nc.vector.reduce_sum(
                    zg[:], eg[:].rearrange("p (c j) -> p c j", j=J), axis=AX.X
                )
                zr = zrp.tile([128, CPH], F32, tag="zr", name=f"zr{h}")
                nc.vector.reciprocal(zr[:], zg[:])
                cg = cijp.tile([128, CPH * J], BF16, tag="cg", name=f"cg{h}")
                nc.vector.tensor_tensor(
                    cg[:].rearrange("p (c j) -> p c j", j=J),
                    eg[:].rearrange("p (c j) -> p c j", j=J),
                    zr[:].unsqueeze(2).broadcast_to([128, CPH, J]),
                    op=ALU.mult,
                )
                for cl in range(CPH):
                    c = h * CPH + cl
                    off = CPH * 128 + cl * E1
                    nc.tensor.matmul(
                        psg[ts(b, J), :],
                        cg[:, ts(cl, J)],
                        pks[h][:, off : off + DIN],
                        start=(c % CPB == 0),
                        stop=(c % CPB == CPB - 1),
                        tile_position=(0, J * b),
                    )

            for h in range(NH):
                emit_logits(h)
                if h >= LAG:
                    emit_chain(h - LAG)
            for h in range(NH - LAG, NH):
                emit_chain(h)

            gout = sbt.tile([128, DIN], F32, tag="gout", name="gout")
            nc.scalar.copy(gout[:], psg[:])
            nc.sync.dma_start(o_d.ap(), gout[:])

    nc.compile()
    return nc


def _get(name):
    if name not in _CACHE:
        if name == "p1":
            _CACHE[name] = _build_phase1()
        else:
            _CACHE[name] = _build_phase2()
    return _CACHE[name]


def kernel(u, W):
    import ml_dtypes

    bf16 = ml_dtypes.bfloat16
    u = np.ascontiguousarray(u, dtype=np.float32)
    W = np.ascontiguousarray(W, dtype=np.float32)
    W0 = np.ascontiguousarray(W[0])  # [128, 512]
    ub = u.astype(bf16)

    # padded re-blocked layout: u1[i][p, ((b,c),e)] = [u[4i+b, 32p+c, :] | e_b]
    up = np.zeros((B, N, E1), dtype=bf16)
    up[:, :, :DIN] = ub
    for b in range(B_LOC):
        up[b::B_LOC, :, DIN + b] = 1.0  # batch index within the core shard
    up = up.reshape(N_CORES, B_LOC, 128, CPB, E1).transpose(0, 2, 1, 3, 4)
    u1 = [np.ascontiguousarray(up[i].reshape(128, B_LOC * CPB * E1))
          for i in range(N_CORES)]
    # transposed copy with the same row permutation:
    # ut[i][d, b*4096 + c*128 + p] = u[4i+b, 32p+c, d]
    ut3 = ub.reshape(N_CORES, B_LOC, 128, CPB, DIN).transpose(0, 4, 1, 3, 2)
    # phase-2 packed layout: per half-batch h, [ut_h | u1_h]
    pk = np.empty((N_CORES, 128, NH, PKW), dtype=bf16)
    utv = ut3.reshape(N_CORES, 128, NH, CPH * 128)
    u1v = up.reshape(N_CORES, 128, NH, CPH * E1)
    pk[:, :, :, : CPH * 128] = utv
    pk[:, :, :, CPH * 128 :] = u1v
    pks = [np.ascontiguousarray(pk[i].reshape(128, NH * PKW))
           for i in range(N_CORES)]

    # ---- phase 1: per-core Gram + rowsums ----
    nc1 = _get("p1")
    r1 = run_bass_kernel_spmd(
        nc1,
        [{"u1": u1[i]} for i in range(N_CORES)],
        core_ids=list(range(N_CORES)),
        trace=PROFILE,
    )
    if PROFILE:
        LAST_TIMES["phase1_ns"] = r1.exec_time_ns

    # ---- host: global scalar reduction (the "all-reduce" of 3 scalars) ----
    C = np.zeros((128, 128), dtype=np.float64)
    Rall = np.empty((128, B), dtype=np.float64)
    for i in range(N_CORES):
        p = r1.results[i]["p1"].astype(np.float64)
        C += p[:, :DIN]
        Rall[:, i * B_LOC : (i + 1) * B_LOC] = p[:, DIN:E1]
    W0d = W0.astype(np.float64)
    M = W0d @ W0d.T
    S2 = float(np.vdot(M, C))
    T = Rall.T @ W0d  # [B, 512]
    S1 = float(T.sum())
    s = S1 / np.sqrt(max(S2, 1e-12))
    sjh2 = (s / J) * T
    n2 = float((sjh2 * sjh2).sum())
    sj2 = (sjh2 / np.sqrt(max(n2, 1e-12))).reshape(B, J, D)
    # A[b][din, j] = sum_dd W0[din, j*16+dd] * sj2[b, j, dd];  fold s in
    A = np.einsum("dje,bje->bdj", W0d.reshape(DIN, J, D), sj2)
    As = (s * A).astype(bf16)  # [B, 128, 32]

    # ---- phase 2: logits/softmax/G ----
    nc2 = _get("p2")
    in2 = [
        {
            "pk": pks[i],
            "A": np.ascontiguousarray(
                As[i * B_LOC : (i + 1) * B_LOC].transpose(1, 0, 2).reshape(DIN, -1)
            ),
        }
        for i in range(N_CORES)
    ]
    r2 = run_bass_kernel_spmd(
        nc2, in2, core_ids=list(range(N_CORES)), trace=PROFILE
    )
    if PROFILE:
        LAST_TIMES["phase2_ns"] = r2.exec_time_ns

    # ---- host: tiny fold + squash (O(B*J*D*DIN)) ----
    G = np.concatenate(
        [r2.results[i]["out"].astype(np.float64).reshape(B_LOC, J, DIN)
         for i in range(N_CORES)]
    )  # [B, J, 128]
    sjh3 = s * np.einsum("bjd,dje->bje", G, W0d.reshape(DIN, J, D))
    s2 = (sjh3 * sjh3).sum(axis=-1, keepdims=True) + 1e-7
    out = (np.sqrt(s2) / (1.0 + s2)) * sjh3
    return out.astype(np.float32)


# revision 16
# speedup vs baseline: 1.1405x; 1.0344x over previous
"""Trainium2 Bass kernel for nn_Capsule (dynamic routing capsule layer).

Math: with cij initialized to zeros, routing iteration 1 collapses to
cij = 1/32 (softmax of zeros), so the whole forward reduces to:
  T[b,j,d]   = sum_n u_hat[b,j,n,d]            (= rowsum(u[b]) @ W)
  S1         = sum(u_hat) = sum(T)
  S2         = sum(u_hat^2) = <W W^T, u^T u>   (feature Gram)
  s          = S1 * rsqrt(max(S2, 1e-12))      (global l2_normalize scalar)
  sjh2       = (s/32) * T ; sj2 = sjh2 * rsqrt(max(sum(sjh2^2), 1e-12))
  logits     = s * (u @ A[b]),  A[b][din,j] = sum_dd W[din,(j,dd)] sj2[b,j,dd]
  cij        = softmax_j(logits)
  G[b][j,:]  = sum_n cij[b,j,n] u[b,n,:]
  out        = squash(s * (G[b] fold W))
u_hat (256 MiB) is never materialized.  Sharding: data-parallel over
batch B (4 per core).  Cross-core reduction (Gram + rowsums -> 3
scalars) and the tiny O(B*J*D*DIN) fold/squash run on the host between
the two launches (in-kernel collectives cost ~65us here, far above the
two-launch overhead).

DMA layout: u rows are re-blocked on the host so each SBUF partition's
DRAM source is one contiguous run (row r of a batch lands at partition
r//32, chunk r%32).  Each row is padded to 132 with a one-hot batch
indicator so a single accumulating matmul chain yields both the Gram
(cols 0:128) and per-batch rowsums (cols 128:132).  The Gram and both
phase-2 matmuls are permutation-invariant to this row order (softmax
rows are independent; the transposed copy uses the same permutation).
Matmul operands are bf16 (fp32 accumulation in PSUM, rel err ~4e-3).
"""

import numpy as np

import concourse.bacc as bacc
import concourse.mybir as mybir
import concourse.tile as tile
from concourse.bass import ts
from concourse.bass_utils import run_bass_kernel_spmd

N_CORES = 8
B, N, DIN = 32, 4096, 128
J, D = 32, 16
K = J * D  # 512
B_LOC = B // N_CORES          # 4 batches per core
CPB = N // 128                # 32 chunks of 128 rows per batch
E1 = DIN + B_LOC              # 132: row + one-hot batch indicator
NH = 2 * B_LOC                # 8 half-batch groups
CPH = CPB // 2                # 16 chunks per half-batch
F32 = mybir.dt.float32
BF16 = mybir.dt.bfloat16
AX = mybir.AxisListType
ALU = mybir.AluOpType
ACTF = mybir.ActivationFunctionType

PROFILE = False
LAST_TIMES = {}

_CACHE = {}


def _new_bass():
    return bacc.Bacc(
        "TRN2",
        target_bir_lowering=False,
        debug=False,
        enable_asserts=False,
        num_devices=N_CORES,
    )


def _build_phase1():
    """Per core: one accumulating matmul chain over 128 row-chunks of
    the padded u layout -> [C | R] = [128, 132] (Gram + per-batch
    rowsums)."""
    nc = _new_bass()
    u_d = nc.dram_tensor("u1", [128, B_LOC * CPB * E1], BF16, kind="ExternalInput")
    o_d = nc.dram_tensor("p1", [128, E1], F32, kind="ExternalOutput")

    with tile.TileContext(nc) as tc:
        with (
            tc.tile_pool(name="upool", bufs=1) as upool,
            tc.tile_pool(name="psp", bufs=1, space="PSUM") as psp,
            tc.tile_pool(name="sbp", bufs=1) as sbp,
        ):
            # 8 half-batch DMAs on the two HWDGE rings; each partition's
            # DRAM source is one contiguous 16*132*2 B run.
            ugs = []
            for h in range(NH):
                ug = upool.tile([128, CPH * E1], BF16, tag=f"ug{h}", name=f"ug{h}")
                ugs.append(ug)
                eng = nc.sync if h % 2 == 0 else nc.scalar
                eng.dma_start(ug[:], u_d.ap()[:, ts(h, CPH * E1)])

            acc = psp.tile([128, E1], F32, tag="acc", name="acc")
            for c in range(B_LOC * CPB):
                h, cl = divmod(c, CPH)
                view = ugs[h][:].rearrange("p (c e) -> p c e", e=E1)[:, cl, :]
                nc.tensor.matmul(
                    acc[:],
                    view[:, 0:DIN],
                    view,
                    start=(c == 0),
                    stop=(c == B_LOC * CPB - 1),
                )

            outsb = sbp.tile([128, E1], F32, tag="outsb", name="outsb")
            nc.scalar.copy(outsb[:], acc[:])
            nc.sync.dma_start(o_d.ap(), outsb[:])

    nc.compile()
    return nc


CPQ = 8                      # chunks per quarter-batch
NQ = 4 * B_LOC               # 16 quarters
QW = CPQ * 128 + CPQ * E1    # 2080: [ut_q | u1_q] packed quarter width


def _build_phase2():
    """Per core: logits -> softmax -> G accumulation; returns G."""
    nc = _new_bass()
    # packed per quarter-batch: cols [0:1024] = transposed u, cols
    # [1024:2080] = natural u rows padded to 132
    u_d = nc.dram_tensor("pk", [128, NQ * QW], BF16, kind="ExternalInput")
    a_d = nc.dram_tensor("A", [DIN, B_LOC * J], BF16, kind="ExternalInput")  # s*A
    # out row 32*b+j holds G[b, j, :] (length-128 din)
    o_d = nc.dram_tensor("out", [128, DIN], F32, kind="ExternalOutput")

    with tile.TileContext(nc) as tc:
        with (
            tc.tile_pool(name="const", bufs=1) as cstp,
            tc.tile_pool(name="upool", bufs=1) as upool,
            tc.tile_pool(name="expp", bufs=4) as expp,
            tc.tile_pool(name="cijp", bufs=4) as cijp,
            tc.tile_pool(name="zgp", bufs=3) as zgp,
            tc.tile_pool(name="zrp", bufs=3) as zrp,
            tc.tile_pool(name="sbt", bufs=1) as sbt,
            tc.tile_pool(name="plp", bufs=4, space="PSUM") as plp,
            tc.tile_pool(name="tlp", bufs=1, space="PSUM") as tlp,
        ):
            # small load first so it doesn't queue behind the u loads
            a_sb = cstp.tile([128, B_LOC * J], BF16, tag="a_sb", name="a_sb")
            nc.sync.dma_start(a_sb[:], a_d.ap())

            # front quarters arrive as 1.04 MiB pairs, the tail as
            # single quarters (shorter post-DMA chain tail).  ACT gets
            # only the first two pairs so its sequencer isn't stuck in
            # ring-full DIRECT2D issue when the exp chain starts; sync
            # (no compute) carries the rest.
            qtile = [None] * NQ   # quarter -> (tile, col offset)
            for t in range(6):
                pk = upool.tile([128, 2 * QW], BF16, tag=f"pp{t}", name=f"pp{t}")
                eng = nc.scalar if t < 2 else nc.sync
                eng.dma_start(pk[:], u_d.ap()[:, ts(t, 2 * QW)])
                qtile[2 * t] = (pk, 0)
                qtile[2 * t + 1] = (pk, QW)
            for q in range(12, NQ):
                pk = upool.tile([128, QW], BF16, tag=f"ps{q}", name=f"ps{q}")
                nc.sync.dma_start(pk[:], u_d.ap()[:, ts(q, QW)])
                qtile[q] = (pk, 0)

            psg = tlp.tile([128, DIN], F32, tag="psg", name="psg")  # G accumulator

            pls = [None] * NQ
            # quarters of logits emitted ahead of their chain: DMA
            # arrivals are ~1.5-3us apart and a chain takes ~2us after
            # its quarter lands; larger LAG makes chain q's G matmuls
            # wait (via the static PE stream) for logits q+LAG's DMA.
            LAG = 2

            def emit_logits(q):
                b = q // 4
                tile_, off = qtile[q]
                pls[q] = plp.tile([128, CPQ * J], F32, tag="pl", name=f"pl{q}")
                for cl in range(CPQ):
                    nc.tensor.matmul(
                        pls[q][:, ts(cl, J)],
                        tile_[:, off + cl * 128 : off + (cl + 1) * 128],
                        a_sb[:, ts(b, J)],
                        start=True,
                        stop=True,
                    )

            def emit_chain(q):
                # softmax over j (free axis) + G matmuls for quarter q
                b = q // 4
                tile_, off = qtile[q]
                uoff = off + CPQ * 128
                eg = expp.tile([128, CPQ * J], F32, tag="eg", name=f"eg{q}")
                nc.scalar.activation(eg[:], pls[q][:], ACTF.Exp)
                zg = zgp.tile([128, CPQ], F32, tag="zg", name=f"zg{q}")
                nc.vector.reduce_sum(
                    zg[:], eg[:].rearrange("p (c j) -> p c j", j=J), axis=AX.X
                )
                zr = zrp.tile([128, CPQ], F32, tag="zr", name=f"zr{q}")
                nc.vector.reciprocal(zr[:], zg[:])
                cg = cijp.tile([128, CPQ * J], BF16, tag="cg", name=f"cg{q}")
                nc.vector.tensor_tensor(
                    cg[:].rearrange("p (c j) -> p c j", j=J),
                    eg[:].rearrange("p (c j) -> p c j", j=J),
                    zr[:].unsqueeze(2).broadcast_to([128, CPQ, J]),
                    op=ALU.mult,
                )
                for cl in range(CPQ):
                    c = q * CPQ + cl
                    nc.tensor.matmul(
                        psg[ts(b, J), :],
                        cg[:, ts(cl, J)],
                        tile_[:, uoff + cl * E1 : uoff + cl * E1 + DIN],
                        start=(c % CPB == 0),
                        stop=(c % CPB == CPB - 1),
                        tile_position=(0, J * b),
                    )

            for q in range(NQ):
                emit_logits(q)
                if q >= LAG:
                    emit_chain(q - LAG)
            for q in range(NQ - LAG, NQ):
                emit_chain(q)

            gout = sbt.tile([128, DIN], F32, tag="gout", name="gout")
            nc.scalar.copy(gout[:], psg[:])
            nc.sync.dma_start(o_d.ap(), gout[:])

    nc.compile()
    return nc


def _get(name):
    if name not in _CACHE:
        if name == "p1":
            _CACHE[name] = _build_phase1()
        else:
            _CACHE[name] = _build_phase2()
    return _CACHE[name]


def kernel(u, W):
    import ml_dtypes

    bf16 = ml_dtypes.bfloat16
    u = np.ascontiguousarray(u, dtype=np.float32)
    W = np.ascontiguousarray(W, dtype=np.float32)
    W0 = np.ascontiguousarray(W[0])  # [128, 512]
    ub = u.astype(bf16)

    # padded re-blocked layout: u1[i][p, ((b,c),e)] = [u[4i+b, 32p+c, :] | e_b]
    up = np.zeros((B, N, E1), dtype=bf16)
    up[:, :, :DIN] = ub
    for b in range(B_LOC):
        up[b::B_LOC, :, DIN + b] = 1.0  # batch index within the core shard
    up = up.reshape(N_CORES, B_LOC, 128, CPB, E1).transpose(0, 2, 1, 3, 4)
    u1 = [np.ascontiguousarray(up[i].reshape(128, B_LOC * CPB * E1))
          for i in range(N_CORES)]
    # transposed copy with the same row permutation:
    # ut[i][d, b*4096 + c*128 + p] = u[4i+b, 32p+c, d]
    ut3 = ub.reshape(N_CORES, B_LOC, 128, CPB, DIN).transpose(0, 4, 1, 3, 2)
    # phase-2 packed layout: per quarter-batch q, [ut_q | u1_q]
    pk = np.empty((N_CORES, 128, NQ, QW), dtype=bf16)
    utv = ut3.reshape(N_CORES, 128, NQ, CPQ * 128)
    u1v = up.reshape(N_CORES, 128, NQ, CPQ * E1)
    pk[:, :, :, : CPQ * 128] = utv
    pk[:, :, :, CPQ * 128 :] = u1v
    pks = [np.ascontiguousarray(pk[i].reshape(128, NQ * QW))
           for i in range(N_CORES)]

    # ---- phase 1: per-core Gram + rowsums ----
    nc1 = _get("p1")
    r1 = run_bass_kernel_spmd(
        nc1,
        [{"u1": u1[i]} for i in range(N_CORES)],
        core_ids=list(range(N_CORES)),
        trace=PROFILE,
    )
    if PROFILE:
        LAST_TIMES["phase1_ns"] = r1.exec_time_ns

    # ---- host: global scalar reduction (the "all-reduce" of 3 scalars) ----
    C = np.zeros((128, 128), dtype=np.float64)
    Rall = np.empty((128, B), dtype=np.float64)
    for i in range(N_CORES):
        p = r1.results[i]["p1"].astype(np.float64)
        C += p[:, :DIN]
        Rall[:, i * B_LOC : (i + 1) * B_LOC] = p[:, DIN:E1]
    W0d = W0.astype(np.float64)
    M = W0d @ W0d.T
    S2 = float(np.vdot(M, C))
    T = Rall.T @ W0d  # [B, 512]
    S1 = float(T.sum())
    s = S1 / np.sqrt(max(S2, 1e-12))
    sjh2 = (s / J) * T
    n2 = float((sjh2 * sjh2).sum())
    sj2 = (sjh2 / np.sqrt(max(n2, 1e-12))).reshape(B, J, D)
    # A[b][din, j] = sum_dd W0[din, j*16+dd] * sj2[b, j, dd];  fold s in
    A = np.einsum("dje,bje->bdj", W0d.reshape(DIN, J, D), sj2)
    As = (s * A).astype(bf16)  # [B, 128, 32]

    # ---- phase 2: logits/softmax/G ----
    nc2 = _get("p2")
    in2 = [
        {
            "pk": pks[i],
            "A": np.ascontiguousarray(
                As[i * B_LOC : (i + 1) * B_LOC].transpose(1, 0, 2).reshape(DIN, -1)
            ),
        }
        for i in range(N_CORES)
    ]
    r2 = run_bass_kernel_spmd(
        nc2, in2, core_ids=list(range(N_CORES)), trace=PROFILE
    )
    if PROFILE:
        LAST_TIMES["phase2_ns"] = r2.exec_time_ns

    # ---- host: tiny fold + squash (O(B*J*D*DIN)) ----
    G = np.concatenate(
        [r2.results[i]["out"].astype(np.float64).reshape(B_LOC, J, DIN)
         for i in range(N_CORES)]
    )  # [B, J, 128]
    sjh3 = s * np.einsum("bjd,dje->bje", G, W0d.reshape(DIN, J, D))
    s2 = (sjh3 * sjh3).sum(axis=-1, keepdims=True) + 1e-7
    out = (np.sqrt(s2) / (1.0 + s2)) * sjh3
    return out.astype(np.float32)
